# revision 1
# baseline (speedup 1.0000x reference)
"""Trainium2 Bass kernel for nn_CombLinearTCQ (trellis-coded-quantized linear).

out = x @ W.T where W rows are decoded from bitshift-trellis streams via a
512x2 lookup table. Sharding: out_features split 8 ways (rows of W), x
replicated; each core decodes its 512 W rows on-device (DVE bit extraction +
native pool-buffer GATHER for the table lookup), then runs the matmul.
"""
import numpy as np

import concourse.bass as bass
import concourse.tile as tile
from concourse import mybir
from concourse.bass_utils import run_bass_kernel_spmd
import concourse.bass_interp as _bass_interp

# The Tile scheduler's no-exec interpreter doesn't know the raw GATHER /
# POOL_BUFFER_LOAD opcodes we emit; treat them as opaque (deps are declared
# via ins/outs on the InstISA).
_orig_visit_isa = _bass_interp._visit_InstISA

def _visit_isa_tolerant(isa, instruction, sim):
    try:
        return _orig_visit_isa(isa, instruction, sim)
    except NotImplementedError:
        if instruction.isa_opcode in (
            isa.Opcode.NEURON_ISA_TPB_OPCODE_GATHER.value,
            isa.Opcode.NEURON_ISA_TPB_OPCODE_POOL_BUFFER_LOAD.value,
        ):
            return None
        raise

_bass_interp._visit_InstISA = _visit_isa_tolerant

# problem constants (hardcoded per harness contract)
B, IN_F, OUT_F = 8192, 4096, 4096
NCORES = 8
MPC = OUT_F // NCORES          # 512 out-features per core
TD = 16                        # trellis tile is 16x16
KV = (4, 2)
NW = (32, 16)                  # packed 16-bit words per tile stream
NCHUNK = 4                     # m-chunks of 128 rows per core (2 kv4 + 2 kv2)
KSPAN = 1024                   # k processed per decode step
NSPAN = IN_F // KSPAN          # 4
NKB = IN_F // 128              # 32 k-blocks for matmul
NIC = B // 512                 # 16 batch chunks of 512


def _word_maps():
    """Per (chunk-kind kv, partition p%16=r, k-position) word index + shift."""
    maps = {}
    for kv in (4, 2):
        nw = 32 if kv == 4 else 16
        # word index (within extended row) and shift for each (r, c)
        widx = np.zeros((16, 16), np.int32)
        shm1 = np.zeros(16, np.int32)
        for c in range(16):
            u = c // 2
            if kv == 4:
                delta = [0, 0, 0, 1, 1, 1, 1, 2][u]
                j = 4 * u + 7 - 16 * delta
            else:
                delta = [0, 0, 0, 0, 0, 1, 1, 1][u]
                j = 2 * u + 7 - 16 * delta
            shm1[c] = 23 - j - 1
            for r in range(16):
                base = (2 * r) if kv == 4 else r
                widx[r, c] = base + delta
        maps[kv] = (widx, shm1, nw)
    return maps


def _host_prepare(inp, trellis1, trellis2, tlut):
    xt = np.ascontiguousarray(inp.T)                      # [IN_F, B]
    t1e = np.concatenate([trellis1, trellis1[:, :2]], 1)  # [32768, 34]
    t2e = np.concatenate([trellis2, trellis2[:, :2]], 1)  # [32768, 18]
    tlf = np.ascontiguousarray(tlut.reshape(1, -1))       # [1, 1024]
    maps = _word_maps()

    kt_of_k = np.arange(IN_F) // 16                       # [4096]
    c_of_k = np.arange(IN_F) % 16

    per_core = []
    sh_full = {}
    for kv in (4, 2):
        widx, shm1, _ = maps[kv]
        sh_full[kv] = shm1[c_of_k].astype(np.int32)       # [4096]
    par_full = (c_of_k % 2).astype(np.int32)

    shp = np.stack([sh_full[4][:16], sh_full[2][:16]], 0)  # [2, 16]
    parr = par_full[:16].reshape(1, -1)                     # [1, 16]

    for c in range(NCORES):
        was, wbs = [], []
        for q in range(NCHUNK):
            kv = 4 if q < 2 else 2
            te = t1e if q < 2 else t2e
            widx, _, _ = maps[kv]
            mt0 = 16 * c + 8 * (q % 2)
            p = np.arange(128)
            mt = mt0 + p // 16                            # [128]
            r = p % 16
            tau = mt[:, None] * 256 + kt_of_k[None, :]    # [128, 4096]
            w = widx[r[:, None], c_of_k[None, :]]         # [128, 4096]
            was.append(te[tau, w])
            wbs.append(te[tau, w + 1])
        wa = np.ascontiguousarray(np.stack(was, 0).astype(np.int32))  # [4,128,4096]
        wb = np.ascontiguousarray(np.stack(wbs, 0).astype(np.int32))
        per_core.append({
            "xt": xt, "tlf": tlf, "wa": wa, "wb": wb,
            "shp": shp, "parr": parr,
            "ident": np.eye(128, dtype=np.float32),
        })
    return per_core


def _build(use_f32r=True):
    nc = bass.Bass(target_bir_lowering=False)
    Op = nc.isa.Opcode
    f32 = mybir.dt.float32
    i32 = mybir.dt.int32
    mmdt = mybir.dt.float32r if use_f32r else f32

    xt = nc.dram_tensor("xt", [IN_F, B], f32, kind="ExternalInput")
    tlf = nc.dram_tensor("tlf", [1, 1024], f32, kind="ExternalInput")
    wa = nc.dram_tensor("wa", [NCHUNK, 128, IN_F], i32, kind="ExternalInput")
    wb = nc.dram_tensor("wb", [NCHUNK, 128, IN_F], i32, kind="ExternalInput")
    shp = nc.dram_tensor("shp", [2, 16], i32, kind="ExternalInput")
    parr = nc.dram_tensor("parr", [1, 16], i32, kind="ExternalInput")
    ident = nc.dram_tensor("ident", [128, 128], f32, kind="ExternalInput")
    ot = nc.dram_tensor("ot", [MPC, B], f32, kind="ExternalOutput")

    with (
        nc.sbuf_tensor("tab", [128, 1024], f32) as tab,
        nc.sbuf_tensor("idx0", [128, KSPAN], i32) as idx0,
        nc.sbuf_tensor("idx1", [128, KSPAN], i32) as idx1,
        nc.sbuf_tensor("gat0", [128, KSPAN], f32) as gat0,
        nc.sbuf_tensor("gat1", [128, KSPAN], f32) as gat1,
    ):
        tab_addr = nc.lookup_mloc("tab").addr
        idx_addr = [nc.lookup_mloc("idx0").addr, nc.lookup_mloc("idx1").addr]
        gat_addr = [nc.lookup_mloc("gat0").addr, nc.lookup_mloc("gat1").addr]
        idxb = [idx0, idx1]
        gatb = [gat0, gat1]

        with tile.TileContext(nc) as tc:
            with (
                tc.tile_pool(name="const", bufs=1) as constp,
                tc.tile_pool(name="words", bufs=3) as wordsp,
                tc.tile_pool(name="pairs", bufs=2) as pairsp,
                tc.tile_pool(name="wt", bufs=1) as wtp,
                tc.tile_pool(name="xs", bufs=16) as xsp,
                tc.tile_pool(name="outs", bufs=3) as outsp,
                tc.tile_pool(name="pst", bufs=2, space="PSUM") as pstp,
                tc.tile_pool(name="psm", bufs=1, space="PSUM") as psmp,
            ):
                # --- constants ---
                nc.sync.dma_start(
                    tab[:], tlf.ap().rearrange("a b -> (a b)").unsqueeze(0)
                    .partition_broadcast(128))
                sh_t = constp.tile([128, 32], i32)
                nc.sync.dma_start(
                    sh_t[:], shp.ap().rearrange("a b -> (a b)").unsqueeze(0)
                    .partition_broadcast(128))
                par_t = constp.tile([128, 16], i32)
                nc.sync.dma_start(
                    par_t[:], parr.ap().rearrange("a b -> (a b)").unsqueeze(0)
                    .partition_broadcast(128))
                id_t = constp.tile([128, 128], f32)
                nc.sync.dma_start(id_t[:], ident[:])

                # pool-buffer load of the flat tlut (1024 entries)
                nc.gpsimd.isa(
                    Op.NEURON_ISA_TPB_OPCODE_POOL_BUFFER_LOAD,
                    {"src_mem_pattern": {
                        "start_addr": {"addr_immediate": tab_addr},
                        "step_elem": [1, 0, 0, 0],
                        "num_elem": [1024, 1, 1, 1]},
                     "in_dtype": 10, "num_active_channels": 128,
                     "start_index": 0, "mask": 0x3FF},
                    verify=False,
                    ins=[nc.gpsimd.lower_ap(tab[:])],
                    outs=[nc.gpsimd.lower_ap(tab[:])],
                )

                # --- decode: 4 m-chunks x 4 k-spans ---
                # one WT tile per (m-chunk, k-span) so matmuls can start as
                # soon as their slice is decoded
                wts = {}
                for q in range(NCHUNK):
                    for s in range(NSPAN):
                        wqs = wtp.tile([128, (KSPAN // 128) * 128], mmdt,
                                       tag=f"wt{q}_{s}")
                        wts[(q, s)] = wqs
                step = 0
                for q in range(NCHUNK):
                    kvi = 0 if q < 2 else 1
                    for s in range(NSPAN):
                        buf = step % 2
                        k0 = s * KSPAN
                        a_t = wordsp.tile([128, KSPAN], i32, tag="wa")
                        nc.sync.dma_start(a_t[:], wa.ap()[q, :, k0:k0 + KSPAN])
                        b_t = wordsp.tile([128, KSPAN], i32, tag="wb")
                        nc.sync.dma_start(b_t[:], wb.ap()[q, :, k0:k0 + KSPAN])
                        pair = pairsp.tile([128, KSPAN], i32, tag="pair")
                        nc.vector.tensor_scalar(
                            out=pair[:], in0=a_t[:], scalar1=16, scalar2=None,
                            op0=mybir.AluOpType.logical_shift_left)
                        nc.vector.tensor_tensor(
                            out=pair[:], in0=pair[:], in1=b_t[:],
                            op=mybir.AluOpType.bitwise_or)
                        idxt = idxb[buf]
                        nc.vector.tensor_tensor(
                            out=idxt[:].rearrange("p (a b) -> p a b", b=16),
                            in0=pair[:].rearrange("p (a b) -> p a b", b=16),
                            in1=sh_t[:, kvi * 16:(kvi + 1) * 16].unsqueeze(1)
                            .broadcast_to([128, KSPAN // 16, 16]),
                            op=mybir.AluOpType.logical_shift_right)
                        nc.vector.tensor_scalar(
                            out=idxt[:], in0=idxt[:], scalar1=1022, scalar2=None,
                            op0=mybir.AluOpType.bitwise_and)
                        nc.vector.tensor_tensor(
                            out=idxt[:].rearrange("p (a b) -> p a b", b=16),
                            in0=idxt[:].rearrange("p (a b) -> p a b", b=16),
                            in1=par_t[:].unsqueeze(1)
                            .broadcast_to([128, KSPAN // 16, 16]),
                            op=mybir.AluOpType.add)
                        gt = gatb[buf]
                        nc.gpsimd.isa(
                            Op.NEURON_ISA_TPB_OPCODE_GATHER,
                            {"src_mem_pattern": {
                                "start_addr": {"addr_immediate": idx_addr[buf]},
                                "step_elem": [1, 0, 0, 0],
                                "num_elem": [KSPAN, 1, 1, 1]},
                             "dst_mem_pattern": {
                                "start_addr": {"addr_immediate": gat_addr[buf]},
                                "step_elem": [1, 0, 0, 0],
                                "num_elem": [KSPAN, 1, 1, 1]},
                             "in_dtype": 9, "out_dtype": 10,
                             "num_active_channels": 128,
                             "index_miss_behavior": 0, "free_pool_buffer": 0,
                             "immediate": {"imm_bitvec_uint32": 0}},
                            verify=False,
                            ins=[nc.gpsimd.lower_ap(idxt[:]),
                                 nc.gpsimd.lower_ap(tab[:])],
                            outs=[nc.gpsimd.lower_ap(gt[:])],
                        )
                        # transpose W-natural [m,k] -> WT [k,m]
                        for j in range(KSPAN // 128):
                            kb = s * (KSPAN // 128) + j
                            ps = pstp.tile([128, 128], f32, tag="tps")
                            nc.tensor.transpose(
                                ps[:], gt[:, j * 128:(j + 1) * 128], id_t[:])
                            nc.vector.tensor_copy(
                                wts[(q, s)][:, j * 128:(j + 1) * 128], ps[:])
                        step += 1

                # --- matmul: out.T[m, i] += WT[k, m].T @ xT[k, i] ---
                for ic in range(NIC):
                    pss = []
                    for mt in range(4):
                        pmm = psmp.tile([128, 512], f32, tag=f"mm{mt}")
                        pss.append(pmm)
                    for kb in range(NKB):
                        xti = xsp.tile([128, 512], f32, tag="xt")
                        nc.sync.dma_start(
                            xti[:],
                            xt.ap()[kb * 128:(kb + 1) * 128,
                                    ic * 512:(ic + 1) * 512])
                        if use_f32r:
                            xmm = xsp.tile([128, 512], mmdt, tag="xr")
                            nc.vector.tensor_copy(xmm[:], xti[:])
                        else:
                            xmm = xti
                        for mt in range(4):
                            jj = kb % (KSPAN // 128)
                            nc.tensor.matmul(
                                pss[mt][:],
                                wts[(mt, kb // (KSPAN // 128))][:, jj * 128:(jj + 1) * 128],
                                xmm[:],
                                start=(kb == 0), stop=(kb == NKB - 1))
                    for mt in range(4):
                        ob = outsp.tile([128, 512], f32, tag="ob")
                        nc.scalar.copy(ob[:], pss[mt][:])
                        nc.sync.dma_start(
                            ot.ap()[mt * 128:(mt + 1) * 128,
                                    ic * 512:(ic + 1) * 512],
                            ob[:])
    _split_waits(nc)
    return nc


def _split_waits(nc, maxw=1):
    """Walrus in this toolchain accepts at most one sem wait per instruction;
    move extra waits emitted by Tile's final drain onto inserted drains."""
    n_new = 0
    for fn in nc.m.functions:
        for bb in fn.blocks:
            insts = bb.instructions
            i = 0
            while i < len(insts):
                inst = insts[i]
                si = inst.sync_info
                if si is not None and len(si.on_wait) > maxw:
                    waits = list(si.on_wait)
                    keep = waits[-maxw:]
                    extra = waits[:-maxw]
                    pos = i
                    for j in range(0, len(extra), maxw):
                        d = mybir.InstDrain(
                            name=f"wsplit-{inst.name}-{j}", ins=[], outs=[])
                        d.engine = inst.engine
                        d.sync_info = mybir.SyncInfo(
                            on_wait=extra[j:j + maxw], on_update=[])
                        insts.insert(pos, d)
                        pos += 1
                        i += 1
                        n_new += 1
                    si.on_wait = keep
                    inst.sync_info = si
                i += 1
    return n_new


_NC_CACHE = {}


def kernel(inp, trellis1, trellis2, tlut):
    inp = np.asarray(inp, dtype=np.float32)
    trellis1 = np.asarray(trellis1, dtype=np.int32)
    trellis2 = np.asarray(trellis2, dtype=np.int32)
    tlut = np.asarray(tlut, dtype=np.float32)

    in_maps = _host_prepare(inp, trellis1, trellis2, tlut)
    if "nc" not in _NC_CACHE:
        _NC_CACHE["nc"] = _build()
    nc = _NC_CACHE["nc"]
    res = run_bass_kernel_spmd(nc, in_maps, core_ids=list(range(NCORES)))

    OT = np.empty((OUT_F, B), np.float32)
    for c in range(NCORES):
        otc = res.results[c]["ot"]
        OT[256 * c: 256 * (c + 1)] = otc[:256]
        OT[2048 + 256 * c: 2048 + 256 * (c + 1)] = otc[256:]
    return np.ascontiguousarray(OT.T).astype(inp.dtype)



# revision 2
# speedup vs baseline: 1.1009x; 1.1009x over previous
"""Trainium2 Bass kernel for nn_CombLinearTCQ (trellis-coded-quantized linear).

out = x @ W.T with W decoded on-device from the trellis LUT. Sharding:
out_features split 8 ways, x replicated in fp16.

v4 changes vs v2 (682.6 us measured):
 - decode is pure DMA -> GATHER: host packs the 9-bit trellis codes as uint16
   gather indices (the v2 trace showed the DVE shift/mask chain pacing decode
   at ~3us/strip, starving the PE through the first batch group)
 - gather emits fp16 directly into the resident W^T strip (fp16 pool table)
 - DMA queue split: x tiles on Sync, indices + output on Scalar (the single
   Sync queue was 85% busy in v2, delaying PSUM evictions)
 - x loaded as [128, 1024] tiles; all 8 PSUM banks accumulate one batch group
   (8 matmuls per k-block keeps the PE ahead of the gather stream)
"""
import os
import numpy as np

import concourse.bass as bass
import concourse.tile as tile
from concourse import mybir
from concourse.bass_utils import run_bass_kernel_spmd
import concourse.bass_interp as _bass_interp

# The Tile scheduler's no-exec interpreter doesn't know the raw GATHER /
# POOL_BUFFER_LOAD opcodes we emit; treat them as opaque (deps are declared
# via ins/outs on the InstISA).
_orig_visit_isa = _bass_interp._visit_InstISA


def _visit_isa_tolerant(isa, instruction, sim):
    try:
        return _orig_visit_isa(isa, instruction, sim)
    except NotImplementedError:
        if instruction.isa_opcode in (
            isa.Opcode.NEURON_ISA_TPB_OPCODE_GATHER.value,
            isa.Opcode.NEURON_ISA_TPB_OPCODE_POOL_BUFFER_LOAD.value,
        ):
            return None
        raise


_bass_interp._visit_InstISA = _visit_isa_tolerant

# problem constants (hardcoded per harness contract)
B, IN_F, OUT_F = 8192, 4096, 4096
NCORES = 8
MPC = OUT_F // NCORES          # 512 out-features per core
NKB = IN_F // 128              # 32 k-blocks
NGG = B // 1024                # 8 batch groups of 1024


def _word_maps():
    """Per (kv, k%16) word index + 9-bit-code shift for the 32-bit pair."""
    maps = {}
    for kv in (4, 2):
        widx = np.zeros((16, 16), np.int32)
        s9 = np.zeros(16, np.int32)
        for c in range(16):
            u = c // 2
            if kv == 4:
                delta = [0, 0, 0, 1, 1, 1, 1, 2][u]
                j = 4 * u + 7 - 16 * delta
            else:
                delta = [0, 0, 0, 0, 0, 1, 1, 1][u]
                j = 2 * u + 7 - 16 * delta
            s9[c] = 23 - j
            for r in range(16):
                base = (2 * r) if kv == 4 else r
                widx[r, c] = base + delta
        maps[kv] = (widx, s9)
    return maps


def _host_prepare(inp, trellis1, trellis2, tlut):
    xh = inp.T.astype(np.float16)                         # [IN_F, B] fp16
    t1e = np.concatenate([trellis1, trellis1[:, :2]], 1)  # [32768, 34]
    t2e = np.concatenate([trellis2, trellis2[:, :2]], 1)  # [32768, 18]
    maps = _word_maps()

    kt_of_k = np.arange(IN_F) // 16
    c_of_k = np.arange(IN_F) % 16

    def codes_for(te, widx, s9):
        rows = np.arange(2048)
        mt = rows // 16
        r = rows % 16
        tau = mt[:, None] * 256 + kt_of_k[None, :]        # [2048, 4096]
        w = widx[r[:, None], c_of_k[None, :]]             # [2048, 4096]
        A = te[tau, w].astype(np.uint32)
        Bw = te[tau, w + 1].astype(np.uint32)
        pair = (A << np.uint32(16)) | (Bw & np.uint32(0xFFFF))
        sh = s9[c_of_k].astype(np.uint32)                 # [4096]
        return ((pair >> sh[None, :]) & np.uint32(511)).astype(np.uint16)

    widx4, s9_4 = maps[4]
    widx2, s9_2 = maps[2]
    codes1 = codes_for(t1e, widx4, s9_4)                  # [2048, 4096] u16
    codes2 = codes_for(t2e, widx2, s9_2)

    p128 = np.arange(128)
    tabpo = np.ascontiguousarray(tlut.T[p128 % 2]).astype(np.float16)  # [128, 512]

    per_core = []
    for c in range(NCORES):
        blk = np.concatenate(
            [codes1[256 * c: 256 * (c + 1)], codes2[256 * c: 256 * (c + 1)]], 0
        )                                                  # [512 m, 4096 k]
        idx = np.ascontiguousarray(blk.T).reshape(NKB, 128, MPC)
        per_core.append({"xt": xh, "idx": idx, "tab": tabpo})
    return per_core


def _build():
    nc = bass.Bass(target_bir_lowering=False)
    Op = nc.isa.Opcode
    f32 = mybir.dt.float32
    f16 = mybir.dt.float16
    u16 = mybir.dt.uint16

    xt = nc.dram_tensor("xt", [IN_F, B], f16, kind="ExternalInput")
    idxd = nc.dram_tensor("idx", [NKB, 128, MPC], u16, kind="ExternalInput")
    tab = nc.dram_tensor("tab", [128, 512], f16, kind="ExternalInput")
    ot = nc.dram_tensor("ot", [B, MPC], f32, kind="ExternalOutput")

    with (
        nc.sbuf_tensor("tabs", [128, 512], f16) as tabs,
        nc.sbuf_tensor("idx0", [128, MPC], u16) as idx0,
        nc.sbuf_tensor("idx1", [128, MPC], u16) as idx1,
        nc.sbuf_tensor("idx2", [128, MPC], u16) as idx2,
        nc.sbuf_tensor("idx3", [128, MPC], u16) as idx3,
        nc.sbuf_tensor("wtb", [128, NKB * MPC], f16) as wtb,
    ):
        tab_addr = nc.lookup_mloc("tabs").addr
        idx_addr = [nc.lookup_mloc(f"idx{i}").addr for i in range(4)]
        wtb_addr = nc.lookup_mloc("wtb").addr
        idxb = [idx0, idx1, idx2, idx3]

        with tile.TileContext(nc) as tc:
            with (
                tc.tile_pool(name="xs", bufs=6) as xsp,
                tc.tile_pool(name="outs", bufs=6) as outsp,
                tc.tile_pool(name="psm", bufs=1, space="PSUM") as psmp,
            ):
                # --- constants (Scalar queue) ---
                nc.scalar.dma_start(tabs[:], tab[:])

                # pool-buffer load of the per-partition parity tlut (512 fp16)
                nc.gpsimd.isa(
                    Op.NEURON_ISA_TPB_OPCODE_POOL_BUFFER_LOAD,
                    {"src_mem_pattern": {
                        "start_addr": {"addr_immediate": tab_addr},
                        "step_elem": [1, 0, 0, 0],
                        "num_elem": [512, 1, 1, 1]},
                     "in_dtype": 7, "num_active_channels": 128,
                     "start_index": 0, "mask": 0x1FF},
                    verify=False,
                    ins=[nc.gpsimd.lower_ap(tabs[:])],
                    outs=[nc.gpsimd.lower_ap(tabs[:])],
                )

                # --- decode: DMA u16 codes, gather fp16 into W^T strips ---
                for kb in range(NKB):
                    buf = kb % 4
                    idxt = idxb[buf]
                    nc.scalar.dma_start(idxt[:], idxd.ap()[kb])
                    nc.gpsimd.isa(
                        Op.NEURON_ISA_TPB_OPCODE_GATHER,
                        {"src_mem_pattern": {
                            "start_addr": {"addr_immediate": idx_addr[buf]},
                            "step_elem": [1, 0, 0, 0],
                            "num_elem": [MPC, 1, 1, 1]},
                         "dst_mem_pattern": {
                            "start_addr": {"addr_immediate":
                                           wtb_addr + kb * MPC * 2},
                            "step_elem": [1, 0, 0, 0],
                            "num_elem": [MPC, 1, 1, 1]},
                         "in_dtype": 5, "out_dtype": 7,
                         "num_active_channels": 128,
                         "index_miss_behavior": 0, "free_pool_buffer": 0,
                         "immediate": {"imm_bitvec_uint32": 0}},
                        verify=False,
                        ins=[nc.gpsimd.lower_ap(idxt[:]),
                             nc.gpsimd.lower_ap(tabs[:])],
                        outs=[nc.gpsimd.lower_ap(
                            wtb[:, kb * MPC:(kb + 1) * MPC])],
                    )

                # --- matmul: psum[b=128, m=512] = sum_k x[b,k] W^T[k,m] ---
                for g in range(NGG):
                    pss = [psmp.tile([128, MPC], f32, tag=f"mm{q}",
                                     name=f"ps{q}") for q in range(8)]
                    for kb in range(NKB):
                        xti = xsp.tile([128, 1024], f16, tag="xt")
                        nc.sync.dma_start(
                            xti[:],
                            xt.ap()[kb * 128:(kb + 1) * 128,
                                    g * 1024:(g + 1) * 1024])
                        for q in range(8):
                            nc.tensor.matmul(
                                pss[q][:],
                                xti[:, q * 128:(q + 1) * 128],
                                wtb[:, kb * MPC:(kb + 1) * MPC],
                                start=(kb == 0), stop=(kb == NKB - 1))
                    for q in range(8):
                        ob = outsp.tile([128, MPC], f32, tag="ob")
                        nc.scalar.copy(ob[:], pss[q][:])
                        nc.gpsimd.dma_start(
                            ot.ap()[(g * 8 + q) * 128:(g * 8 + q + 1) * 128, :],
                            ob[:])
    _split_waits(nc)
    return nc


def _split_waits(nc, maxw=1):
    """Walrus in this toolchain accepts at most one sem wait per instruction;
    move extra waits emitted by Tile's final drain onto inserted drains."""
    n_new = 0
    for fn in nc.m.functions:
        for bb in fn.blocks:
            insts = bb.instructions
            i = 0
            while i < len(insts):
                inst = insts[i]
                si = inst.sync_info
                if si is not None and len(si.on_wait) > maxw:
                    waits = list(si.on_wait)
                    keep = waits[-maxw:]
                    extra = waits[:-maxw]
                    pos = i
                    for j in range(0, len(extra), maxw):
                        d = mybir.InstDrain(
                            name=f"wsplit-{inst.name}-{j}", ins=[], outs=[])
                        d.engine = inst.engine
                        d.sync_info = mybir.SyncInfo(
                            on_wait=extra[j:j + maxw], on_update=[])
                        insts.insert(pos, d)
                        pos += 1
                        i += 1
                        n_new += 1
                    si.on_wait = keep
                    inst.sync_info = si
                i += 1
    return n_new


_NC_CACHE = {}
_LAST = {}


def kernel(inp, trellis1, trellis2, tlut):
    inp = np.asarray(inp, dtype=np.float32)
    trellis1 = np.asarray(trellis1, dtype=np.int32)
    trellis2 = np.asarray(trellis2, dtype=np.int32)
    tlut = np.asarray(tlut, dtype=np.float32)

    in_maps = _host_prepare(inp, trellis1, trellis2, tlut)
    if "nc" not in _NC_CACHE:
        _NC_CACHE["nc"] = _build()
    nc = _NC_CACHE["nc"]
    res = run_bass_kernel_spmd(nc, in_maps, core_ids=list(range(NCORES)))
    _LAST["res"] = res

    out = np.empty((B, OUT_F), np.float32)
    for c in range(NCORES):
        otc = res.results[c]["ot"]
        out[:, 256 * c: 256 * (c + 1)] = otc[:, :256]
        out[:, 2048 + 256 * c: 2048 + 256 * (c + 1)] = otc[:, 256:]
    return out
